# revision 1
# baseline (speedup 1.0000x reference)
"""Trainium2 Bass kernel for MimiAttention (GQA + RoPE + causal softmax).

Problem: B=2, S=2048, H=1024, NH=16 q-heads, NKV=4 kv-heads, HD=64.
Sharding: 8 cores = 2 (batch) x 4 (kv-group).  Each core computes one batch's
attention for one GQA group (4 q-heads sharing 1 kv head) and the partial
o-projection for those heads; the host sums the 4 partials per batch.

Per-core device pipeline (all matmuls bf16 in / fp32 psum out):
  1. QKV projection in [d, s] layout.  RoPE is realized without any
     cross-partition shuffles by computing a second projection with
     sign-permuted weight rows (W2 rows: d<32 -> -W[d+32], d>=32 -> W[d-32]):
       q_rot = q*cos + q2*sin
     The scores contraction then uses the 128-dim identity
       q_rot . k_rot = concat(q*cos, q2*sin) . concat(k_rot, k_rot)
     so Qhat = [q*cos; q2*sin] needs only ONE elementwise multiply per chunk,
     and Khat = [k_rot; k_rot] is built by one matmul with the fold matrix
     J[p,m] = (p == m mod 64).
  2. Scores computed TRANSPOSED (scoresT[j,i]) per key-tile, exp on ACT with
     the 1/sqrt(64) scale folded in (no max subtraction needed: |s*scale|<~3),
     causal zeroing via one gpsimd affine_select per (head, j-tile).
  3. attnV: out[i, d] with lhsT = expT tile, rhs = [v | ones]: column 64 gives
     the softmax denominator as a per-partition scalar -> reciprocal + scale.
  4. o-projection after PE-transposing attn [i,c] -> [c,i]; output written
     transposed ([h, s]); host transposes back and sums partials.
"""

import numpy as np
import ml_dtypes

B, S, H = 2, 2048, 1024
NH, NKV, HD = 16, 4, 64
G = NH // NKV            # 4 q-heads per kv head
THETA = 10000.0
N_CORES = 8

BF16 = ml_dtypes.bfloat16


def _build_nc():
    import concourse.mybir as mybir
    import concourse.tile as tile
    from concourse.tile import add_dep_helper
    from concourse import bacc

    f32 = mybir.dt.float32
    bf16 = mybir.dt.bfloat16

    nc = bacc.Bacc("TRN2", target_bir_lowering=False)

    xTd = nc.dram_tensor("xT", [H, S], bf16, kind="ExternalInput")
    wqkd = nc.dram_tensor("wqkT", [H, 640], bf16, kind="ExternalInput")
    wvd = nc.dram_tensor("wvT", [H, HD], bf16, kind="ExternalInput")
    csd = nc.dram_tensor("cs", [128, S], bf16, kind="ExternalInput")
    wod = nc.dram_tensor("woT", [G * HD, H], bf16, kind="ExternalInput")
    djd = nc.dram_tensor("dupJ", [128, 128], bf16, kind="ExternalInput")
    idd = nc.dram_tensor("ident", [128, 128], bf16, kind="ExternalInput")
    trid = nc.dram_tensor("trimask", [128, 128], bf16, kind="ExternalInput")
    oTd = nc.dram_tensor("oT", [H, S], bf16, kind="ExternalOutput")

    NSB = S // 512        # 4 chunks of 512
    NST = S // 128        # 16 tiles of 128
    KC = H // 128         # 8 contraction chunks
    scale = float(1.0 / np.sqrt(HD))

    with tile.TileContext(nc) as tc:
        import contextlib
        ctx = contextlib.ExitStack()
        with ctx:
            consts = ctx.enter_context(tc.tile_pool(name="consts", bufs=1))
            acts = ctx.enter_context(tc.tile_pool(name="acts", bufs=1))
            anp = ctx.enter_context(tc.tile_pool(name="attn", bufs=1))
            rcp = ctx.enter_context(tc.tile_pool(name="rcp", bufs=6))
            etp = ctx.enter_context(tc.tile_pool(name="etri", bufs=4))
            ep = ctx.enter_context(tc.tile_pool(name="exps", bufs=1))
            otp = ctx.enter_context(tc.tile_pool(name="ot", bufs=8))
            # Shared PSUM pool: tag "s" [128,1024] fp32 x 2 slots (4 banks)
            # used by qkv waves, k-fold, v-proj, scores and the o-projection.
            psp = ctx.enter_context(
                tc.tile_pool(name="ps", bufs=4, space="PSUM"))
            pav = ctx.enter_context(
                tc.tile_pool(name="ps_av", bufs=1, space="PSUM"))
            pvp = ctx.enter_context(
                tc.tile_pool(name="ps_v", bufs=1, space="PSUM"))

            # ---- input DMAs: xt kc0 first, then weights/tables, then the
            # rest of xt (full 2048-col rows keep DMA descriptors large).
            xt_sb = consts.tile([128, KC, S], bf16, tag="xt")
            nc.sync.dma_start(xt_sb[:, 0, :], xTd[0:128, :])
            wqk_sb = consts.tile([128, KC, 640], bf16, tag="wqk")
            nc.sync.dma_start(wqk_sb, wqkd.rearrange("(kc p) m -> p kc m", p=128))
            dj_sb = consts.tile([128, 128], bf16, tag="dj")
            nc.sync.dma_start(dj_sb, djd[:, :])
            wv_sb = consts.tile([128, KC, HD], bf16, tag="wv")
            nc.sync.dma_start(wv_sb, wvd.rearrange("(kc p) m -> p kc m", p=128))
            tri_sb = consts.tile([128, 128], bf16, tag="tri")
            nc.sync.dma_start(tri_sb, trid[:, :])
            cs_sb = consts.tile([128, S], bf16, tag="cs")
            nc.sync.dma_start(cs_sb, csd[:, :])
            for kc in range(1, KC):
                nc.sync.dma_start(xt_sb[:, kc, :],
                                  xTd[kc * 128:(kc + 1) * 128, :])
            id_sb = consts.tile([128, 128], bf16, tag="id")
            nc.sync.dma_start(id_sb, idd[:, :])
            wo_sb = consts.tile([128, 2, H], bf16, tag="wo")
            nc.sync.dma_start(wo_sb, wod.rearrange("(kc p) m -> p kc m", p=128))

            qhat = [acts.tile([128, S], bf16, tag=f"qh{m}", name=f"qhat{m}")
                    for m in range(G)]
            khat = acts.tile([128, S], bf16, tag="khat")
            ktmp = acts.tile([128, S], bf16, tag="ktmp")
            v_sb = acts.tile([128, NST, HD + 1], bf16, tag="vsb")
            attn_n = [anp.tile([128, G * HD], bf16, tag=f"an{it}",
                               name=f"attn{it}")
                      for it in range(NST)]
            expT = [ep.tile([128, S], bf16, tag=f"e{jt}", name=f"expT{jt}")
                    for jt in range(NST)]
            aT = [acts.tile([128, S], bf16, tag=f"aT{c}", name=f"aTc{c}")
                  for c in range(2)]

            # attnV accumulators: slice `it` = bank[it//7][:, (it%7)*65 :+65]
            avb = [pav.tile([128, w], f32, tag=f"av{b}", name=f"avb{b}")
                   for b, w in ((0, 455), (1, 455), (2, 130))]

            def av_slice(it):
                b, o = it // 7, (it % 7) * 65
                return avb[b][:, o:o + 65]

            def proj_chunk(m, dst, n, off_slot=False):
                col = n * 512
                if off_slot:
                    ps = pvp.tile([128, 512], f32, tag="v", name="psw")
                else:
                    ps = psp.tile([128, 512], f32, tag="s", name="psw")
                for kc in range(KC):
                    nc.tensor.matmul(
                        ps, wqk_sb[:, kc, m * 128:(m + 1) * 128],
                        xt_sb[:, kc, col:col + 512],
                        start=(kc == 0), stop=(kc == KC - 1))
                nc.vector.tensor_mul(
                    dst[:, col:col + 512], ps, cs_sb[:, col:col + 512])

            def proj_wave(m, dst):
                for n in range(NSB):
                    proj_chunk(m, dst, n)

            # ---- k-side and q0 waves interleaved: both are paced by the
            # same xt DMA stream, so let them share the slot pipeline; the
            # k-fold for chunk n trails its kk2 chunk immediately.
            nc.vector.memset(v_sb[:, :, HD:HD + 1], 1.0)
            for n in range(NSB):
                proj_chunk(G, ktmp, n)
                psf = pvp.tile([128, 512], f32, tag="v", name="psf")
                nc.tensor.matmul(psf, dj_sb, ktmp[:, n * 512:(n + 1) * 512],
                                 start=True, stop=True)
                nc.vector.tensor_copy(khat[:, n * 512:(n + 1) * 512], psf)
                proj_chunk(0, qhat[0], n)

            def v_proj(st):
                psv = pvp.tile([128, HD], f32, tag="v", name="psv")
                for kc in range(KC):
                    nc.tensor.matmul(
                        psv, xt_sb[:, kc, st * 128:(st + 1) * 128],
                        wv_sb[:, kc, :],
                        start=(kc == 0), stop=(kc == KC - 1))
                nc.vector.tensor_copy(v_sb[:, st, 0:HD], psv)

            v_proj(0)

            # ---- transpose + o-projection, streamed per 512-col chunk ----
            def oproj_group(nchunk, shared=False):
                for it in range(nchunk * 4, nchunk * 4 + 4):
                    for c in range(2):
                        psx = psp.tile([128, 128], bf16, tag="s", name="pst")
                        nc.tensor.transpose(
                            psx, attn_n[it][:, c * 128:(c + 1) * 128], id_sb)
                        nc.vector.tensor_copy(
                            aT[c][:, it * 128:(it + 1) * 128], psx)
                col = nchunk * 512
                for hc in range(KC):
                    if shared or hc % 2 == 0:
                        ps2 = psp.tile([128, 512], f32, tag="s", name="pso2")
                    else:
                        ps2 = pvp.tile([128, 512], f32, tag="v", name="pso2")
                    for kc2 in range(2):
                        nc.tensor.matmul(
                            ps2, wo_sb[:, kc2, hc * 128:(hc + 1) * 128],
                            aT[kc2][:, col:col + 512],
                            start=(kc2 == 0), stop=(kc2 == 1))
                    ot = otp.tile([128, 512], bf16, tag="ot", name="otst")
                    if shared and hc % 2 == 1:
                        nc.scalar.copy(ot, ps2)
                    else:
                        nc.vector.tensor_copy(ot, ps2)
                    nc.sync.dma_start(
                        oTd[hc * 128:(hc + 1) * 128, col:col + 512], ot)

            # ---- attention: per head, per key-tile ----
            for h in range(G):
                bank_first = {}
                for jt in range(NST):
                    if h < G - 1 and jt in (1, 4, 7, 10):
                        proj_chunk(h + 1, qhat[h + 1], (jt - 1) // 3,
                                   off_slot=(h > 0))
                    lo = jt * 128
                    lhsT = khat[:, jt * 128:(jt + 1) * 128]
                    for ic in range(NSB):
                        cs_, ce = ic * 512, (ic + 1) * 512
                        if ce <= lo:
                            continue
                        s0 = max(cs_, lo)
                        ps = psp.tile([128, 512], f32, tag="s", name="pss")
                        nc.tensor.matmul(
                            ps[:, s0 - cs_:512], lhsT,
                            qhat[h][:, s0:ce], start=True, stop=True)
                        nc.scalar.activation(
                            expT[jt][:, s0:ce], ps[:, s0 - cs_:512],
                            mybir.ActivationFunctionType.Exp, scale=scale)
                    # causal triangle mask for the diagonal block (gpsimd,
                    # off the PE/ACT critical path)
                    etri = etp.tile([128, 128], bf16, tag="et", name="etri")
                    nc.gpsimd.tensor_mul(etri, expT[jt][:, lo:lo + 128],
                                         tri_sb)

                    # attnV: descending it so the masked diagonal tile is
                    # needed last; first matmul of each bank per head uses
                    # start=True (clears the bank has_written bits), all
                    # others accumulate / per-element overwrite.
                    for it in range(NST - 1, jt - 1, -1):
                        lhs = (etri if it == jt
                               else expT[jt][:, it * 128:(it + 1) * 128])
                        b = it // 7
                        first = jt == 0 and b not in bank_first
                        mm = nc.tensor.matmul(
                            av_slice(it), lhs, v_sb[:, jt, :],
                            start=first, stop=(it == jt),
                            skip_group_check=True)
                        if first:
                            bank_first[b] = mm
                        elif jt == 0:
                            add_dep_helper(mm.ins, bank_first[b].ins,
                                           sync=False,
                                           reason="bank clear first")

                    # slice it=jt is complete: normalize
                    pso = av_slice(jt)
                    rc = rcp.tile([128, 1], f32, tag="rc", name="rc")
                    nc.vector.reciprocal(rc, pso[:, HD:HD + 1])
                    nc.vector.tensor_scalar_mul(
                        attn_n[jt][:, h * HD:(h + 1) * HD], pso[:, 0:HD], rc)
                    if h == 0 and jt < NST - 1:
                        v_proj(jt + 1)
                    if h == G - 1 and jt % 4 == 3 and jt < NST - 1:
                        oproj_group(jt // 4)
            oproj_group(NSB - 1, shared=True)

    nc.finalize()
    return nc


def _host_inputs(hidden_states, position_ids, wq, wk, wv, wo):
    """Build the 8 per-core input maps."""
    def w2_of(w):
        # w: [64, H] rows of one head; returns sign-permuted rows
        w2 = np.empty_like(w)
        w2[:32] = -w[32:64]
        w2[32:] = w[:32]
        return w2

    dupJ = np.zeros((128, 128), np.float32)
    for p in range(128):
        dupJ[p, p % 64] = 1.0
        dupJ[p, p % 64 + 64] = 1.0
    dupJ = dupJ.astype(BF16)
    ident = np.eye(128, dtype=np.float32).astype(BF16)
    trimask = np.triu(np.ones((128, 128), np.float32)).astype(BF16)

    in_maps = []
    for core in range(N_CORES):
        b, kv = core // NKV, core % NKV
        xT = np.ascontiguousarray(hidden_states[b].T).astype(BF16)

        cols = []
        for i in range(G):
            h = kv * G + i
            wqh = wq[h * HD:(h + 1) * HD]
            cols.append(wqh.T)
            cols.append(w2_of(wqh).T)
        wkh = wk[kv * HD:(kv + 1) * HD]
        cols.append(wkh.T)
        cols.append(w2_of(wkh).T)
        wqkT = np.ascontiguousarray(np.concatenate(cols, axis=1)).astype(BF16)

        wvT = np.ascontiguousarray(wv[kv * HD:(kv + 1) * HD].T).astype(BF16)
        woT = np.ascontiguousarray(
            wo[:, kv * G * HD:(kv + 1) * G * HD].T).astype(BF16)

        inv = 1.0 / (THETA ** (np.arange(0, HD, 2, dtype=np.float32) / HD))
        freqs = position_ids[b].astype(np.float32)[:, None] * inv[None, :]
        emb = np.concatenate([freqs, freqs], axis=-1)       # [S, 64]
        cs = np.concatenate([np.cos(emb).T, np.sin(emb).T], axis=0)  # [128, S]
        cs = np.ascontiguousarray(cs).astype(BF16)

        in_maps.append({
            "xT": xT, "wqkT": wqkT, "wvT": wvT, "cs": cs, "woT": woT,
            "dupJ": dupJ, "ident": ident, "trimask": trimask,
        })
    return in_maps


_NC_CACHE = {}


def run_cores(in_maps, trace=False, trace_kwargs=None):
    from concourse.bass_utils import run_bass_kernel_spmd
    if "nc" not in _NC_CACHE:
        _NC_CACHE["nc"] = _build_nc()
    nc = _NC_CACHE["nc"]
    return run_bass_kernel_spmd(
        nc, in_maps, core_ids=list(range(N_CORES)),
        trace=trace, **(trace_kwargs or {}))


def kernel(hidden_states, attention_mask, position_ids, wq, wk, wv, wo):
    hidden_states = np.asarray(hidden_states, dtype=np.float32)
    position_ids = np.asarray(position_ids)
    wq = np.asarray(wq, dtype=np.float32)
    wk = np.asarray(wk, dtype=np.float32)
    wv = np.asarray(wv, dtype=np.float32)
    wo = np.asarray(wo, dtype=np.float32)

    in_maps = _host_inputs(hidden_states, position_ids, wq, wk, wv, wo)
    res = run_cores(in_maps)

    out = np.zeros((B, S, H), np.float32)
    for core in range(N_CORES):
        b = core // NKV
        out[b] += res.results[core]["oT"].T.astype(np.float32)
    return out

